# revision 1
# baseline (speedup 1.0000x reference)
"""LocallyConnected2d Trainium2 kernel.

y[b,o,h,w] = sum_{i,ky,kx} x[b,i,h+ky-1,w+kx-1] * weight[i,o,h,w,ky,kx] + bias[o,h,w]

Shapes: x [64,64,32,32], weight [64,64,32,32,3,3], bias [64,32,32] -> y [64,64,32,32].

Strategy
--------
Spatial sharding over H_out: 8 cores x 4 output rows each (receptive fields
need rows h-1..h+4 of x, packed per-core on host).

Per output location (h,w): a K=576 x M=64(cout) x N=64(batch) matmul,
executed as 5 PSUM-accumulating matmuls: 4 chunks of K=128 (each chunk = two
kernel offsets x 64 cin) + 1 tail chunk of K=64 (offset (2,2)).

A K=128 chunk spans two kernel offsets whose x data must appear at the SAME
free-dim offset on partitions 0-63 and 64-127. We pre-shift the bottom copy on
host: X1 has the bottom half shifted by 1 (serves pairs (ky,0)+(ky,1)), X34 is
shifted by 34 (serves pair (0,2)+(1,2)). Offsets are in units of 64-batch
blocks over the flattened (row, col) slab of the padded x slice.

All inputs are host-packed into exact per-core SBUF images so every DMA is a
plain contiguous [P, F] load.
"""

import sys

sys.path.insert(0, "/opt/trn_rl_repo")

import numpy as np

B, CIN, COUT, H, W = 64, 64, 64, 32, 32
K = 3
HOUT, WOUT = 32, 32
NCORES = 8
ROWS = HOUT // NCORES  # output rows per core
SLAB_R = ROWS + 2      # x rows needed per core (halo)
SLAB_C = W + 2         # padded width
RC = SLAB_R * SLAB_C   # flattened (row, col) length

# chunk pairing: j=0..3 -> (ky0,kx0)+(ky1,kx1); tail = (2,2)
PAIRS = [((0, 0), (0, 1)), ((1, 0), (1, 1)), ((2, 0), (2, 1)), ((0, 2), (1, 2))]
TAIL = (2, 2)

_nc_cache = {}


def _build_bass():
    import concourse.bass as bass
    import concourse.tile as tile
    from concourse import bacc, mybir

    f32 = mybir.dt.float32
    nc = bacc.Bacc(None, target_bir_lowering=False)

    x1_d = nc.dram_tensor("x1", (128, RC, B), f32, kind="ExternalInput")
    x34_d = nc.dram_tensor("x34", (128, RC, B), f32, kind="ExternalInput")
    wmain_d = nc.dram_tensor("wmain", (ROWS, 128, WOUT, 4, COUT), f32, kind="ExternalInput")
    wtail_d = nc.dram_tensor("wtail", (ROWS, 64, WOUT, COUT), f32, kind="ExternalInput")
    bias_d = nc.dram_tensor("bias", (ROWS, COUT, WOUT), f32, kind="ExternalInput")
    out_d = nc.dram_tensor("out", (ROWS, COUT, WOUT, B), f32, kind="ExternalOutput")

    with tile.TileContext(nc) as tc:
        with (
            tc.tile_pool(name="xpool", bufs=1) as xpool,
            tc.tile_pool(name="wpool", bufs=2) as wpool,
            tc.tile_pool(name="opool", bufs=2) as opool,
            tc.tile_pool(name="bpool", bufs=1) as bpool,
            tc.tile_pool(name="psum", bufs=8, space=bass.MemorySpace.PSUM) as psum,
        ):
            x1 = xpool.tile([128, RC, B], f32, tag="x1")
            x34 = xpool.tile([128, RC, B], f32, tag="x34")
            nc.sync.dma_start(x1[:], x1_d[:])
            nc.sync.dma_start(x34[:], x34_d[:])

            bi = bpool.tile([COUT, ROWS, WOUT], f32, tag="bias")
            nc.sync.dma_start(
                bi[:], bias_d.rearrange("h o w -> o h w")
            )

            for h in range(ROWS):
                wm = wpool.tile([128, WOUT, 4, COUT], f32, tag="wm")
                wt = wpool.tile([64, WOUT, COUT], f32, tag="wt")
                nc.sync.dma_start(wm[:], wmain_d[h])
                nc.sync.dma_start(wt[:], wtail_d[h])
                ot = opool.tile([COUT, WOUT, B], f32, tag="out")

                for w in range(WOUT):
                    ps = psum.tile([COUT, B], f32, tag="ps")
                    for j, ((ky0, kx0), _) in enumerate(PAIRS):
                        xsrc = x34 if j == 3 else x1
                        rc = (h + ky0) * SLAB_C + (w + kx0)
                        nc.tensor.matmul(
                            ps[:],
                            wm[:, w, j, :],
                            xsrc[:, rc, :],
                            start=(j == 0),
                            stop=False,
                        )
                    rc_t = (h + TAIL[0]) * SLAB_C + (w + TAIL[1])
                    nc.tensor.matmul(
                        ps[:],
                        wt[:, w, :],
                        x1[0:64, rc_t, :],
                        start=False,
                        stop=True,
                    )
                    nc.any.tensor_scalar_add(ot[:, w, :], ps[:], bi[:, h, w : w + 1])

                nc.sync.dma_start(out_d[h], ot[:])

    nc.compile()
    return nc


def get_nc():
    if "nc" not in _nc_cache:
        _nc_cache["nc"] = _build_bass()
    return _nc_cache["nc"]


def _shift(s, d):
    """s: [64, RC, B]; returns s advanced by d blocks along axis 1, zero-filled."""
    out = np.zeros_like(s)
    out[:, : RC - d, :] = s[:, d:, :]
    return out


def pack_inputs(x, weight, bias):
    """Returns list of per-core in_maps (numpy, C-contiguous)."""
    x = np.asarray(x, dtype=np.float32)
    weight = np.asarray(weight, dtype=np.float32)
    bias = np.asarray(bias, dtype=np.float32)

    # padded x: [B, CIN, H+2, W+2]
    xp = np.zeros((B, CIN, H + 2, W + 2), dtype=np.float32)
    xp[:, :, 1:-1, 1:-1] = x

    # weight -> [h, w, ky, kx, cin, cout]
    wt = np.ascontiguousarray(np.transpose(weight, (2, 3, 4, 5, 0, 1)))

    ky0s = np.array([p[0][0] for p in PAIRS])
    kx0s = np.array([p[0][1] for p in PAIRS])
    ky1s = np.array([p[1][0] for p in PAIRS])
    kx1s = np.array([p[1][1] for p in PAIRS])

    in_maps = []
    for c in range(NCORES):
        h0 = c * ROWS
        # x slab rows h0-1 .. h0+ROWS (SLAB_R rows of padded x)
        slab = xp[:, :, h0 : h0 + SLAB_R, :]  # [B, CIN, SLAB_R, SLAB_C]
        s = np.transpose(slab, (1, 2, 3, 0)).reshape(CIN, RC, B)  # [cin, rc, b]
        x1 = np.concatenate([s, _shift(s, 1)], axis=0)
        x34 = np.concatenate([s, _shift(s, 34)], axis=0)

        wh = wt[h0 : h0 + ROWS]  # [ROWS, w, ky, kx, cin, cout]
        top = wh[:, :, ky0s, kx0s]  # [ROWS, w, j, cin, cout]
        bot = wh[:, :, ky1s, kx1s]
        # -> [ROWS, cin, w, j, cout]
        top = np.transpose(top, (0, 3, 1, 2, 4))
        bot = np.transpose(bot, (0, 3, 1, 2, 4))
        wmain = np.concatenate([top, bot], axis=1)  # [ROWS, 128, w, j, cout]
        wtail = np.transpose(wh[:, :, TAIL[0], TAIL[1]], (0, 2, 1, 3))  # [ROWS, cin, w, cout]

        bi = np.transpose(bias[:, h0 : h0 + ROWS, :], (1, 0, 2))  # [ROWS, cout, w]

        in_maps.append(
            {
                "x1": np.ascontiguousarray(x1),
                "x34": np.ascontiguousarray(x34),
                "wmain": np.ascontiguousarray(wmain),
                "wtail": np.ascontiguousarray(wtail),
                "bias": np.ascontiguousarray(bi),
            }
        )
    return in_maps


def unpack_outputs(results):
    """results: list of per-core out_maps with 'out' [ROWS, COUT, WOUT, B]."""
    full = np.concatenate([np.asarray(r["out"]) for r in results], axis=0)
    # [HOUT, COUT, WOUT, B] -> [B, COUT, HOUT, WOUT]
    return np.ascontiguousarray(np.transpose(full, (3, 1, 0, 2)))


def run(in_maps, **kwargs):
    from concourse import bass_utils

    nc = get_nc()
    return bass_utils.run_bass_kernel_spmd(
        nc, in_maps, core_ids=list(range(NCORES)), **kwargs
    )


def kernel(x, weight, bias):
    in_maps = pack_inputs(x, weight, bias)
    res = run(in_maps)
    return unpack_outputs(res.results)


if __name__ == "__main__":
    rng = np.random.default_rng(0)
    x = rng.standard_normal((B, CIN, H, W), dtype=np.float32)
    weight = rng.standard_normal((CIN, COUT, HOUT, WOUT, K, K), dtype=np.float32)
    bias = rng.standard_normal((COUT, HOUT, WOUT), dtype=np.float32)
    y = kernel(x, weight, bias)
    print("out", y.shape, y.dtype)



# revision 3
# speedup vs baseline: 1.5432x; 1.5432x over previous
"""LocallyConnected2d Trainium2 kernel (bf16).

y[b,o,h,w] = sum_{i,ky,kx} x[b,i,h+ky-1,w+kx-1] * weight[i,o,h,w,ky,kx] + bias[o,h,w]

Shapes: x [64,64,32,32], weight [64,64,32,32,3,3], bias [64,32,32] -> y [64,64,32,32].

Strategy
--------
Spatial sharding over H_out: 8 cores x 4 output rows each (receptive fields
need rows h-1..h+4 of x, packed per-core on host). All matmul inputs are bf16
(tolerance is 2e-2; bf16 keeps rel err ~3e-3) which quarters PE cycles vs
fp32 and halves HBM traffic. Output is written bf16 and upcast on host.

Per output location (h,w): K=576 x M=64(cout) x N=64(batch), split into
6 PSUM-accumulating matmuls:
  j=0,1,2  K=128 chunks pairing offsets (ky=j,kx=0)+(ky=j,kx=1); the bottom
           64 partitions read the x slab shifted by 1 column (host-packed
           second copy), so one rhs AP serves both offsets.
  g=0,1,2  singles (0,2),(1,2),(2,2): K=64 matmuls on partitions 0-63
           reading the unshifted slab at rc3 + 34*g. This eliminates the
           fp32 baseline's separate 34-shifted x copy (1/3 less x DMA).

Weight DMA (9.4 MB/core bf16) rides the SP HWDGE ring; x and outputs ride
the ACT ring so prefetches never queue behind stores.
"""

import sys

sys.path.insert(0, "/opt/trn_rl_repo")

import numpy as np
import ml_dtypes

BF16 = ml_dtypes.bfloat16

B, CIN, COUT, H, W = 64, 64, 64, 32, 32
K = 3
HOUT, WOUT = 32, 32
NCORES = 8
ROWS = HOUT // NCORES  # output rows per core
SLAB_R = ROWS + 2      # x rows needed per core (halo)
SLAB_C = W + 2         # padded width
RC = SLAB_R * SLAB_C   # flattened (row, col) length

# full chunk pairing: j=0..2 -> (j,0)+(j,1); singles (0,2),(1,2),(2,2)
PAIRS = [((0, 0), (0, 1)), ((1, 0), (1, 1)), ((2, 0), (2, 1))]
SINGLES = [(0, 2), (1, 2), (2, 2)]

_nc_cache = {}


def _build_bass():
    import concourse.bass as bass
    import concourse.tile as tile
    from concourse import bacc, mybir

    f32 = mybir.dt.float32
    bf16 = mybir.dt.bfloat16
    nc = bacc.Bacc(None, target_bir_lowering=False)

    xa_d = nc.dram_tensor("xa", (128, RC, B), bf16, kind="ExternalInput")
    wmain_d = nc.dram_tensor("wmain", (ROWS, 128, WOUT, 3, COUT), bf16, kind="ExternalInput")
    wtail_d = nc.dram_tensor("wtail", (ROWS, 64, WOUT, 3, COUT), bf16, kind="ExternalInput")
    bias_d = nc.dram_tensor("bias", (ROWS, COUT, WOUT), f32, kind="ExternalInput")
    out_d = nc.dram_tensor("out", (ROWS, COUT, WOUT, B), bf16, kind="ExternalOutput")

    with tile.TileContext(nc) as tc:
        with (
            tc.tile_pool(name="xpool", bufs=1) as xpool,
            tc.tile_pool(name="wpool", bufs=2) as wpool,
            tc.tile_pool(name="opool", bufs=2) as opool,
            tc.tile_pool(name="bpool", bufs=1) as bpool,
            tc.tile_pool(name="psum", bufs=8, space=bass.MemorySpace.PSUM) as psum,
        ):
            xa = xpool.tile([128, RC, B], bf16, tag="xa")
            # split the x load so row-0 compute can start before the whole
            # slab lands; x rides the ACT ring, weights the SP ring
            xsplit = 3 * SLAB_C
            nc.scalar.dma_start(xa[:, 0:xsplit, :], xa_d[:, 0:xsplit, :])
            nc.scalar.dma_start(xa[:, xsplit:RC, :], xa_d[:, xsplit:RC, :])

            bi = bpool.tile([COUT, ROWS, WOUT], f32, tag="bias")
            nc.scalar.dma_start(bi[:], bias_d.rearrange("h o w -> o h w"))

            for h in range(ROWS):
                wm = wpool.tile([128, WOUT, 3, COUT], bf16, tag="wm")
                wt = wpool.tile([64, WOUT, 3, COUT], bf16, tag="wt")
                nc.sync.dma_start(wm[:], wmain_d[h])
                nc.sync.dma_start(wt[:], wtail_d[h])
                ot = opool.tile([COUT, WOUT, B], bf16, tag="out")

                for w in range(WOUT):
                    ps = psum.tile([COUT, B], f32, tag="ps")
                    for j, ((ky0, kx0), _) in enumerate(PAIRS):
                        rc = (h + ky0) * SLAB_C + (w + kx0)
                        nc.tensor.matmul(
                            ps[:],
                            wm[:, w, j, :],
                            xa[:, rc, :],
                            start=(j == 0),
                            stop=False,
                        )
                    rc3 = h * SLAB_C + (w + 2)
                    for g in range(3):
                        nc.tensor.matmul(
                            ps[:],
                            wt[:, w, g, :],
                            xa[0:64, rc3 + g * SLAB_C, :],
                            start=False,
                            stop=(g == 2),
                        )
                    nc.any.tensor_scalar_add(ot[:, w, :], ps[:], bi[:, h, w : w + 1])

                nc.scalar.dma_start(out_d[h], ot[:])

    nc.compile()
    return nc


def get_nc():
    if "nc" not in _nc_cache:
        _nc_cache["nc"] = _build_bass()
    return _nc_cache["nc"]


def _shift(s, d):
    """s: [64, RC, B]; returns s advanced by d blocks along axis 1, zero-filled."""
    out = np.zeros_like(s)
    out[:, : RC - d, :] = s[:, d:, :]
    return out


def pack_inputs(x, weight, bias):
    """Returns list of per-core in_maps (numpy, C-contiguous)."""
    x = np.asarray(x, dtype=np.float32)
    weight = np.asarray(weight, dtype=np.float32)
    bias = np.asarray(bias, dtype=np.float32)

    # padded x: [B, CIN, H+2, W+2]
    xp = np.zeros((B, CIN, H + 2, W + 2), dtype=np.float32)
    xp[:, :, 1:-1, 1:-1] = x

    # weight -> [h, w, ky, kx, cin, cout]
    wt = np.ascontiguousarray(np.transpose(weight, (2, 3, 4, 5, 0, 1)))

    ky0s = np.array([p[0][0] for p in PAIRS])
    kx0s = np.array([p[0][1] for p in PAIRS])
    ky1s = np.array([p[1][0] for p in PAIRS])
    kx1s = np.array([p[1][1] for p in PAIRS])
    kys = np.array([s[0] for s in SINGLES])
    kxs = np.array([s[1] for s in SINGLES])

    in_maps = []
    for c in range(NCORES):
        h0 = c * ROWS
        # x slab rows h0-1 .. h0+ROWS (SLAB_R rows of padded x)
        slab = xp[:, :, h0 : h0 + SLAB_R, :]  # [B, CIN, SLAB_R, SLAB_C]
        s = np.transpose(slab, (1, 2, 3, 0)).reshape(CIN, RC, B)  # [cin, rc, b]
        xa = np.concatenate([s, _shift(s, 1)], axis=0)

        wh = wt[h0 : h0 + ROWS]  # [ROWS, w, ky, kx, cin, cout]
        top = wh[:, :, ky0s, kx0s]  # [ROWS, w, j, cin, cout]
        bot = wh[:, :, ky1s, kx1s]
        # -> [ROWS, cin, w, j, cout]
        top = np.transpose(top, (0, 3, 1, 2, 4))
        bot = np.transpose(bot, (0, 3, 1, 2, 4))
        wmain = np.concatenate([top, bot], axis=1)  # [ROWS, 128, w, 3, cout]
        wtail = np.transpose(wh[:, :, kys, kxs], (0, 3, 1, 2, 4))  # [ROWS, cin, w, 3, cout]

        bi = np.transpose(bias[:, h0 : h0 + ROWS, :], (1, 0, 2))  # [ROWS, cout, w]

        in_maps.append(
            {
                "xa": np.ascontiguousarray(xa.astype(BF16)),
                "wmain": np.ascontiguousarray(wmain.astype(BF16)),
                "wtail": np.ascontiguousarray(wtail.astype(BF16)),
                "bias": np.ascontiguousarray(bi),
            }
        )
    return in_maps


def unpack_outputs(results):
    """results: list of per-core out_maps with 'out' [ROWS, COUT, WOUT, B] bf16."""
    full = np.concatenate(
        [np.asarray(r["out"]).astype(np.float32) for r in results], axis=0
    )
    # [HOUT, COUT, WOUT, B] -> [B, COUT, HOUT, WOUT]
    return np.ascontiguousarray(np.transpose(full, (3, 1, 0, 2)))


def run(in_maps, **kwargs):
    from concourse import bass_utils

    nc = get_nc()
    return bass_utils.run_bass_kernel_spmd(
        nc, in_maps, core_ids=list(range(NCORES)), **kwargs
    )


def kernel(x, weight, bias):
    in_maps = pack_inputs(x, weight, bias)
    res = run(in_maps)
    return unpack_outputs(res.results)


if __name__ == "__main__":
    rng = np.random.default_rng(0)
    x = rng.standard_normal((B, CIN, H, W), dtype=np.float32)
    weight = rng.standard_normal((CIN, COUT, HOUT, WOUT, K, K), dtype=np.float32)
    bias = rng.standard_normal((COUT, HOUT, WOUT), dtype=np.float32)
    y = kernel(x, weight, bias)
    print("out", y.shape, y.dtype)


# revision 5
# speedup vs baseline: 1.8376x; 1.1908x over previous
"""LocallyConnected2d Trainium2 kernel (bf16).

y[b,o,h,w] = sum_{i,ky,kx} x[b,i,h+ky-1,w+kx-1] * weight[i,o,h,w,ky,kx] + bias[o,h,w]

Shapes: x [64,64,32,32], weight [64,64,32,32,3,3], bias [64,32,32] -> y [64,64,32,32].

Strategy
--------
Spatial sharding over H_out: 8 cores x 4 output rows each (receptive fields
need rows h-1..h+4 of x, packed per-core on host). All matmul inputs are bf16
(tolerance is 2e-2; bf16 keeps rel err ~3e-3) which quarters PE cycles vs
fp32 and halves HBM traffic. Output is written bf16 and upcast on host.

Per output location (h,w): K=576 x M=64(cout) x N=64(batch), split into
6 PSUM-accumulating matmuls:
  j=0,1,2  K=128 chunks pairing offsets (ky=j,kx=0)+(ky=j,kx=1); the bottom
           64 partitions read the x slab shifted by 1 column (host-packed
           second copy), so one rhs AP serves both offsets.
  g=0,1,2  singles (0,2),(1,2),(2,2): K=64 matmuls on partitions 0-63
           reading the unshifted slab at rc3 + 34*g.

Schedule: a burst of junk matmuls warms the PE HAM clock-gate (1.2->2.4 GHz)
while the input DMAs fill SBUF; bias rides first on the ACT ring so psum
drains are never gated on it; all weight DMAs are issued upfront in
half-row chunks (bufs=4) so the SP ring stays saturated.
"""

import sys

sys.path.insert(0, "/opt/trn_rl_repo")

import numpy as np
import ml_dtypes

BF16 = ml_dtypes.bfloat16

B, CIN, COUT, H, W = 64, 64, 64, 32, 32
K = 3
HOUT, WOUT = 32, 32
NCORES = 8
ROWS = HOUT // NCORES  # output rows per core
SLAB_R = ROWS + 2      # x rows needed per core (halo)
SLAB_C = W + 2         # padded width
RC = SLAB_R * SLAB_C   # flattened (row, col) length
RC_LO = 3 * SLAB_C     # first x chunk: slab rows 0-2 (covers h=0)

# full chunk pairing: j=0..2 -> (j,0)+(j,1); singles (0,2),(1,2),(2,2)
PAIRS = [((0, 0), (0, 1)), ((1, 0), (1, 1)), ((2, 0), (2, 1))]
SINGLES = [(0, 2), (1, 2), (2, 2)]

N_WARM = 48        # junk matmuls to warm the PE clock gate during DMA fill
WARM_N = 512       # their moving free dim

_nc_cache = {}


def _build_bass():
    import concourse.bass as bass
    import concourse.tile as tile
    from concourse import bacc, mybir

    f32 = mybir.dt.float32
    bf16 = mybir.dt.bfloat16
    nc = bacc.Bacc(None, target_bir_lowering=False)

    xa_d = nc.dram_tensor("xa", (128, RC, B), bf16, kind="ExternalInput")
    wmain_d = nc.dram_tensor("wmain", (ROWS, 128, WOUT, 3, COUT), bf16, kind="ExternalInput")
    wtail_d = nc.dram_tensor("wtail", (ROWS, 64, WOUT, 3, COUT), bf16, kind="ExternalInput")
    bias_d = nc.dram_tensor("bias", (ROWS, COUT, WOUT), f32, kind="ExternalInput")
    out_d = nc.dram_tensor("out", (ROWS, COUT, WOUT, B), bf16, kind="ExternalOutput")

    with tile.TileContext(nc) as tc:
        with (
            tc.tile_pool(name="xpool", bufs=1) as xpool,
            tc.tile_pool(name="wpool", bufs=4) as wpool,
            tc.tile_pool(name="opool", bufs=2) as opool,
            tc.tile_pool(name="bpool", bufs=1) as bpool,
            tc.tile_pool(name="spool", bufs=1) as spool,
            tc.tile_pool(name="psum", bufs=7, space=bass.MemorySpace.PSUM) as psum,
            tc.tile_pool(name="psumw", bufs=1, space=bass.MemorySpace.PSUM) as psumw,
        ):
            # --- PE warm-up: junk matmuls on a zeroed scratch tile keep the
            # HAM activity monitor busy while the real inputs stream in, so
            # the first real matmul runs at 2.4 GHz instead of 1.2.
            scratch = spool.tile([128, WARM_N], bf16, tag="scratch")
            nc.vector.memset(scratch[:], 0)
            wps = psumw.tile([64, WARM_N], f32, tag="warm")
            for i in range(N_WARM):
                nc.tensor.matmul(
                    wps[:], scratch[:, 0:64], scratch[:],
                    start=True, stop=True,
                )

            # bias first on the ACT ring: psum drains depend on it
            bi = bpool.tile([COUT, ROWS, WOUT], f32, tag="bias")
            nc.scalar.dma_start(bi[:], bias_d.rearrange("h o w -> o h w"))

            xa = xpool.tile([128, RC, B], bf16, tag="xa")
            nc.scalar.dma_start(xa[:, 0:RC_LO, :], xa_d[:, 0:RC_LO, :])
            nc.scalar.dma_start(xa[:, RC_LO:RC, :], xa_d[:, RC_LO:RC, :])

            HALF = WOUT // 2
            wms, wts = [], []
            for h in range(ROWS):
                wm = wpool.tile([128, WOUT, 3, COUT], bf16, tag="wm")
                wt = wpool.tile([64, WOUT, 3, COUT], bf16, tag="wt")
                # half-row chunks keep the SP ring busy and let w=0 compute
                # start as soon as the first ~1.2 MB lands
                nc.sync.dma_start(wm[:, 0:HALF], wmain_d[h][:, 0:HALF])
                nc.sync.dma_start(wt[:, 0:HALF], wtail_d[h][:, 0:HALF])
                nc.sync.dma_start(wm[:, HALF:WOUT], wmain_d[h][:, HALF:WOUT])
                nc.sync.dma_start(wt[:, HALF:WOUT], wtail_d[h][:, HALF:WOUT])
                wms.append(wm)
                wts.append(wt)

            for h in range(ROWS):
                wm, wt = wms[h], wts[h]
                ot = opool.tile([COUT, WOUT, B], bf16, tag="out")

                for w in range(WOUT):
                    ps = psum.tile([COUT, B], f32, tag="ps")
                    for j, ((ky0, kx0), _) in enumerate(PAIRS):
                        rc = (h + ky0) * SLAB_C + (w + kx0)
                        nc.tensor.matmul(
                            ps[:],
                            wm[:, w, j, :],
                            xa[:, rc, :],
                            start=(j == 0),
                            stop=False,
                        )
                    rc3 = h * SLAB_C + (w + 2)
                    for g in range(3):
                        nc.tensor.matmul(
                            ps[:],
                            wt[:, w, g, :],
                            xa[0:64, rc3 + g * SLAB_C, :],
                            start=False,
                            stop=(g == 2),
                        )
                    nc.any.tensor_scalar_add(ot[:, w, :], ps[:], bi[:, h, w : w + 1])

                nc.scalar.dma_start(out_d[h], ot[:])

    nc.compile()
    return nc


def get_nc():
    if "nc" not in _nc_cache:
        _nc_cache["nc"] = _build_bass()
    return _nc_cache["nc"]


def _shift(s, d):
    """s: [64, RC, B]; returns s advanced by d blocks along axis 1, zero-filled."""
    out = np.zeros_like(s)
    out[:, : RC - d, :] = s[:, d:, :]
    return out


def pack_inputs(x, weight, bias):
    """Returns list of per-core in_maps (numpy, C-contiguous)."""
    x = np.asarray(x, dtype=np.float32)
    weight = np.asarray(weight, dtype=np.float32)
    bias = np.asarray(bias, dtype=np.float32)

    # padded x: [B, CIN, H+2, W+2]
    xp = np.zeros((B, CIN, H + 2, W + 2), dtype=np.float32)
    xp[:, :, 1:-1, 1:-1] = x

    # weight -> [h, w, ky, kx, cin, cout]
    wt = np.ascontiguousarray(np.transpose(weight, (2, 3, 4, 5, 0, 1)))

    ky0s = np.array([p[0][0] for p in PAIRS])
    kx0s = np.array([p[0][1] for p in PAIRS])
    ky1s = np.array([p[1][0] for p in PAIRS])
    kx1s = np.array([p[1][1] for p in PAIRS])
    kys = np.array([s[0] for s in SINGLES])
    kxs = np.array([s[1] for s in SINGLES])

    in_maps = []
    for c in range(NCORES):
        h0 = c * ROWS
        # x slab rows h0-1 .. h0+ROWS (SLAB_R rows of padded x)
        slab = xp[:, :, h0 : h0 + SLAB_R, :]  # [B, CIN, SLAB_R, SLAB_C]
        s = np.transpose(slab, (1, 2, 3, 0)).reshape(CIN, RC, B)  # [cin, rc, b]
        xa = np.concatenate([s, _shift(s, 1)], axis=0)

        wh = wt[h0 : h0 + ROWS]  # [ROWS, w, ky, kx, cin, cout]
        top = wh[:, :, ky0s, kx0s]  # [ROWS, w, j, cin, cout]
        bot = wh[:, :, ky1s, kx1s]
        # -> [ROWS, cin, w, j, cout]
        top = np.transpose(top, (0, 3, 1, 2, 4))
        bot = np.transpose(bot, (0, 3, 1, 2, 4))
        wmain = np.concatenate([top, bot], axis=1)  # [ROWS, 128, w, 3, cout]
        wtail = np.transpose(wh[:, :, kys, kxs], (0, 3, 1, 2, 4))  # [ROWS, cin, w, 3, cout]

        bi = np.transpose(bias[:, h0 : h0 + ROWS, :], (1, 0, 2))  # [ROWS, cout, w]

        in_maps.append(
            {
                "xa": np.ascontiguousarray(xa.astype(BF16)),
                "wmain": np.ascontiguousarray(wmain.astype(BF16)),
                "wtail": np.ascontiguousarray(wtail.astype(BF16)),
                "bias": np.ascontiguousarray(bi),
            }
        )
    return in_maps


def unpack_outputs(results):
    """results: list of per-core out_maps with 'out' [ROWS, COUT, WOUT, B] bf16."""
    full = np.concatenate(
        [np.asarray(r["out"]).astype(np.float32) for r in results], axis=0
    )
    # [HOUT, COUT, WOUT, B] -> [B, COUT, HOUT, WOUT]
    return np.ascontiguousarray(np.transpose(full, (3, 1, 0, 2)))


def run(in_maps, **kwargs):
    from concourse import bass_utils

    nc = get_nc()
    return bass_utils.run_bass_kernel_spmd(
        nc, in_maps, core_ids=list(range(NCORES)), **kwargs
    )


def kernel(x, weight, bias):
    in_maps = pack_inputs(x, weight, bias)
    res = run(in_maps)
    return unpack_outputs(res.results)


if __name__ == "__main__":
    rng = np.random.default_rng(0)
    x = rng.standard_normal((B, CIN, H, W), dtype=np.float32)
    weight = rng.standard_normal((CIN, COUT, HOUT, WOUT, K, K), dtype=np.float32)
    bias = rng.standard_normal((COUT, HOUT, WOUT), dtype=np.float32)
    y = kernel(x, weight, bias)
    print("out", y.shape, y.dtype)
